# revision 24
# baseline (speedup 1.0000x reference)
"""Trainium2 Bass kernel for AttentionalGraphInteractLayer.

Computes, for x[N,D], adj[N,N], Wf/Wg[D,D], W[D,O] (N=8192, D=O=1024):
    f = x@Wf + bf; g = x@Wg + bg
    scores = where(adj>0, relu(f@g.T), -9e15)
    out = softmax(scores, axis=1) @ (x@W + bW)

Strategy (8 NeuronCores, SPMD):
  - Shard rows of x/adj across cores (1024 rows each). Replicate weights.
  - fp16 single-pass matmuls (PE at 1 cycle/row) for the two big N x N
    stages (scores, attention @ xW) and for xW itself; the f/g projections
    use the 3-pass hi/lo fp16 split because their error feeds the exp.
    Measured rel err vs the oracle: 9.2e-3 (gate 2e-2). All-single-pass
    measured 2.3e-2 -- just over the gate, hence this hybrid.
  - Each core computes fT (kept in SBUF), gT and xW for its row block.
    gT and xW are AllGathered in two 512-token chunks each, interleaved
    with the phase-1 matmuls, so the first half of phase 3 can start
    after only half of each collective has landed. Measured collective
    exposure vs COLL_NONE=1: ~15-30 us.
  - adj ships as fp8e5 {0, -57344}: mask-by-ADD before the exp (masked
    scores sit at -57344, exp underflows to exactly 0), which lets the
    Exp activation's accum_out produce the softmax row-sum directly --
    one DVE op per tile instead of three. bW is folded into xW in phase
    1 (softmax rows sum to 1), so the epilogue is a pure 1/l scale on
    the scalar engine.
  - Row-block flash softmax over 16 j-groups of 512 tokens; the scores
    matmuls of group k+1 are emitted before the softmax/attention of
    group k so the PE static order has no bubble. P.T for the attention
    matmul comes from 4 PE transposes per strip (XBAR DMA transpose was
    tried and is ~2.7x slower end-to-end: ~1.2us/op queue cost).

Budget (per core, 2.4 GHz PE): phase-1 458k cyc (191 us, of which
2x131k is the f/g accuracy tax) + phase-3 1.114M cyc (464 us) = 656 us
PE floor; TimelineSim 689 us; measured ~700 us/invocation on HW.
"""

import os
import numpy as np
import ml_dtypes

import concourse.bass as bass
import concourse.mybir as mybir
import concourse.tile as tile
from concourse import bacc
from concourse.bass_utils import run_bass_kernel_spmd

dt = mybir.dt
AF = mybir.ActivationFunctionType
ALU = mybir.AluOpType

N_CORES = 8
N, D, O = 8192, 1024, 1024
NL = N // N_CORES          # 1024 rows per core
JC = 512                   # j-group token chunk (AllGather granularity)
MASK_NEG = -57344.0        # fp8e5-representable; exp(x-57344) == 0 in fp32

_cache = {}


def _build(sim_single_core=False, reps=1):
    """sim_single_core: build a 1-core variant with collectives replaced by
    local DMA fan-out copies, for TimelineSim cost-model profiling."""
    n_dev = 1 if sim_single_core else N_CORES
    if os.environ.get("COLL_OFF", "0") == "1":
        sim_single_core = True  # timing A/B: fan-out DMA instead of collective
        n_dev = N_CORES
    COLL_NONE = os.environ.get("COLL_NONE", "0") == "1"  # timing floor
    nc = bacc.Bacc("TRN2", target_bir_lowering=False, debug=False,
                   num_devices=n_dev)

    # Per-weight phase-1 matmul pass counts (hi/lo fp16 decomposition):
    # 1 = xh@Wh, 2 = +xl@Wh, 3 = +xh@Wl. f and g feed the exp-sensitive
    # scores matmul, so they get the accurate 3-pass; xW's error is
    # negligible in the output (attention rows sum to 1).
    ph1 = os.environ.get("PH1", "331")
    P1 = {"wf": int(ph1[0]), "wg": int(ph1[1]), "ww": int(ph1[2])}
    need_lo = max(P1.values()) >= 2

    # ---------------- DRAM I/O ----------------
    xT_d = nc.dram_tensor("xT", [D, NL], dt.float16, kind="ExternalInput")
    xT_lo_d = (nc.dram_tensor("xT_lo", [D, NL], dt.float16,
                              kind="ExternalInput") if need_lo else None)
    w_d = {w: nc.dram_tensor(w, [D, D], dt.float16, kind="ExternalInput")
           for w in ("wf", "wg", "ww")}
    wlo_d = {w: nc.dram_tensor(f"{w}_lo", [D, D], dt.float16,
                               kind="ExternalInput")
             for w in ("wf", "wg", "ww") if P1[w] >= 3}
    adj_d = nc.dram_tensor("adj", [NL, N], dt.float8e5, kind="ExternalInput")
    bf_d = nc.dram_tensor("bf", [D, 1], dt.float32, kind="ExternalInput")
    bg_d = nc.dram_tensor("bg", [D, 1], dt.float32, kind="ExternalInput")
    bw_d = nc.dram_tensor("bw", [1, O], dt.float32, kind="ExternalInput")
    out_d = nc.dram_tensor("out", [NL, O], dt.float32, kind="ExternalOutput")

    # collective bounce + gathered buffers, chunked over 512-token halves
    # so phase 3 can start after the first chunk of each gather.
    g_b = [nc.dram_tensor(f"g_b{c}", [D, JC], dt.float16) for c in range(2)]
    xw_b = [nc.dram_tensor(f"xw_b{c}", [JC, O], dt.float16) for c in range(2)]
    g_ag = [nc.dram_tensor(f"g_ag{c}", [N_CORES * D, JC], dt.float16,
                           addr_space="Shared") for c in range(2)]
    xw_ag = [nc.dram_tensor(f"xw_ag{c}", [N_CORES * JC, O], dt.float16,
                            addr_space="Shared") for c in range(2)]

    ident_d = nc.inline_tensor(np.eye(128).astype(np.float16), name="ident16")

    def all_gather(inp, outp):
        if COLL_NONE:
            return
        if sim_single_core:
            for c in range(N_CORES):
                sh = inp.shape[0]
                nc.sync.dma_start(outp[c * sh:(c + 1) * sh, :], inp[:])
        else:
            nc.gpsimd.collective_compute(
                "AllGather", ALU.bypass,
                replica_groups=[list(range(N_CORES))],
                ins=[inp[:]], outs=[outp[:]])

    with tile.TileContext(nc, num_cores=n_dev) as tc:
        # ---- persistent tiles (live for the whole kernel)
        with tc.tile_pool(name="persist", bufs=1) as pp:
            fT = pp.tile([128, 8 * NL], dt.float16, tag="fT")
            ident = pp.tile([128, 128], dt.float16, tag="ident")
            nc.sync.dma_start(ident[:], ident_d[:])
            bw_rep = pp.tile([128, O], dt.float32, tag="bw_rep")
            nc.sync.dma_start(bw_rep[:], bw_d[0:1, :].partition_broadcast(128))
            acc = [pp.tile([128, O], dt.float32, tag=f"acc{s}", name=f"acc{s}")
                   for s in range(8)]
            nm = [pp.tile([128, 2], dt.float32, tag=f"nm{s}", name=f"nm{s}")
                  for s in range(8)]
            lr = [pp.tile([128, 2], dt.float32, tag=f"lr{s}", name=f"lr{s}")
                  for s in range(8)]
            for _rep in range(reps):
                if _rep > 0:
                    # full barrier so reps cannot overlap: the reps>1 builds
                    # exist only to measure per-rep latency honestly
                    tc.strict_bb_all_engine_barrier()
                for s in range(8):
                    nc.gpsimd.memset(acc[s][:], 0.0)
                    nc.gpsimd.memset(nm[s][:], 0.0)
                    nc.gpsimd.memset(lr[s][:], 0.0)

                # ================= phase 1: g/xW/f =================
                # wg and ww interleave at chunk granularity so the four
                # chunked AllGathers fire as early as possible:
                #   g-chunk0 -> AG, xw-chunk0 -> AG, g-chunk1 -> AG,
                #   xw-chunk1 -> AG, then f (local only).
                with tc.tile_pool(name="ph1", bufs=1) as p1, \
                     tc.tile_pool(name="ph1o", bufs=6) as p1o, \
                     tc.tile_pool(name="ph1ps", bufs=4, space="PSUM") as p1ps:
                    # interleave the startup loads d-chunk-first so the first
                    # wg matmuls can begin after ~1/8 of the bytes; ww/wf
                    # loads are emitted later (see below) to stay off the
                    # critical path.
                    xh = p1.tile([128, 8 * NL], dt.float16, tag="xh")
                    xl = None
                    if need_lo:
                        xl = p1.tile([128, 8 * NL], dt.float16, tag="xl",
                                     name="xl")
                    wh, wl = {}, {}
                    for w in ("wg", "ww", "wf"):
                        wh[w] = p1.tile([128, 8 * D], dt.float16,
                                        tag=f"wh_{w}", name=f"wh_{w}")
                        if w in wlo_d:
                            wl[w] = p1.tile([128, 8 * D], dt.float16,
                                            tag=f"wl_{w}", name=f"wl_{w}")

                    def load_w(w, d):
                        nc.sync.dma_start(wh[w][:, d * D:(d + 1) * D],
                                          w_d[w][d * 128:(d + 1) * 128, :])
                        if w in wlo_d:
                            nc.sync.dma_start(wl[w][:, d * D:(d + 1) * D],
                                              wlo_d[w][d * 128:(d + 1) * 128, :])

                    for d in range(8):
                        nc.sync.dma_start(xh[:, d * NL:(d + 1) * NL],
                                          xT_d[d * 128:(d + 1) * 128, :])
                        if need_lo:
                            nc.sync.dma_start(xl[:, d * NL:(d + 1) * NL],
                                              xT_lo_d[d * 128:(d + 1) * 128, :])
                        load_w("wg", d)

                    def mmp(ps, w, wlhs, lslice, rslice):
                        """accumulate x@W passes; wlhs: W on lhsT (g/f) or
                        rhs (xw). Pass 2 adds x_lo, pass 3 adds W_lo."""
                        npass = P1[w]
                        terms = [(wh[w], xh)]
                        if npass >= 2:
                            terms.append((wh[w], xl))
                        if npass >= 3:
                            terms.append((wl[w], xh))
                        last = len(terms) - 1
                        for d in range(8):
                            for ti, (wt, xt) in enumerate(terms):
                                lhs, rhs = ((wt, xt) if wlhs else (xt, wt))
                                nc.tensor.matmul(
                                    ps, lhs[:, lslice(d)], rhs[:, rslice(d)],
                                    start=(d == 0 and ti == 0),
                                    stop=(d == 7 and ti == last))

                    def g_chunk(nck):
                        # gT[dout, tok]: lhsT = Wg[d, dout-blk], rhs = xT[d, tok]
                        for m in range(8):
                            bias_t = p1o.tile([128, 1], dt.float32, tag="bias")
                            nc.sync.dma_start(bias_t[:],
                                              bg_d[m * 128:(m + 1) * 128, :])
                            ps = p1ps.tile([128, JC], dt.float32, tag="ps1")
                            mmp(ps[:], "wg", True,
                                lambda d: slice(d * D + m * 128,
                                                d * D + m * 128 + 128),
                                lambda d: slice(d * NL + nck * JC,
                                                d * NL + nck * JC + JC))
                            hi = p1o.tile([128, JC], dt.float16, tag="hi")
                            nc.scalar.activation(hi[:], ps[:], AF.Identity,
                                                 bias=bias_t[:], scale=1.0)
                            nc.sync.dma_start(
                                g_b[nck][m * 128:(m + 1) * 128, :], hi[:])
                        all_gather(g_b[nck], g_ag[nck])

                    def xw_chunk(nck):
                        # xw[tok, o]: lhsT = xT[d, tok-blk], rhs = W[d, o-chunk]
                        for mm in range(4):
                            m = nck * 4 + mm
                            for oc in range(2):
                                ps = p1ps.tile([128, JC], dt.float32, tag="ps1")
                                mmp(ps[:], "ww", False,
                                    lambda d: slice(d * NL + m * 128,
                                                    d * NL + m * 128 + 128),
                                    lambda d: slice(d * D + oc * JC,
                                                    d * D + oc * JC + JC))
                                hi = p1o.tile([128, JC], dt.float16, tag="hi")
                                # fold bW in here: attention rows sum to 1,
                                # so out = attn@(xW+bW) needs no epilogue add
                                nc.vector.tensor_tensor(
                                    out=hi[:], in0=ps[:],
                                    in1=bw_rep[:, oc * JC:(oc + 1) * JC],
                                    op=ALU.add)
                                nc.sync.dma_start(
                                    xw_b[nck][mm * 128:(mm + 1) * 128,
                                              oc * JC:(oc + 1) * JC], hi[:])
                        all_gather(xw_b[nck], xw_ag[nck])

                    g_chunk(0)
                    for d in range(8):
                        load_w("ww", d)
                    xw_chunk(0)
                    g_chunk(1)
                    for d in range(8):
                        load_w("wf", d)
                    xw_chunk(1)

                    # f stays local in SBUF (transposed: fT[dout, tok])
                    for m in range(8):
                        bias_t = p1o.tile([128, 1], dt.float32, tag="bias")
                        nc.sync.dma_start(bias_t[:],
                                          bf_d[m * 128:(m + 1) * 128, :])
                        for nck in range(2):
                            ps = p1ps.tile([128, JC], dt.float32, tag="ps1")
                            mmp(ps[:], "wf", True,
                                lambda d: slice(d * D + m * 128,
                                                d * D + m * 128 + 128),
                                lambda d: slice(d * NL + nck * JC,
                                                d * NL + nck * JC + JC))
                            nc.scalar.activation(
                                fT[:, m * NL + nck * JC:
                                   m * NL + nck * JC + JC],
                                ps[:], AF.Identity, bias=bias_t[:], scale=1.0)

                # ================= phase 3: flash attention =================
                # 16 j-groups of 512 tokens (cg = source core, half = token
                # half). Software-pipelined: scores of group k+1 are emitted
                # before the softmax/attention of group k.
                with tc.tile_pool(name="gt", bufs=18) as gt_pool, \
                     tc.tile_pool(name="xw", bufs=10) as xw_pool, \
                     tc.tile_pool(name="adj", bufs=4) as adj_pool, \
                     tc.tile_pool(name="work", bufs=3) as wk, \
                     tc.tile_pool(name="tiny", bufs=6) as tiny, \
                     tc.tile_pool(name="ps_sc", bufs=4, space="PSUM") as ps_sc_p, \
                     tc.tile_pool(name="ps_at", bufs=1, space="PSUM") as ps_at_p, \
                     tc.tile_pool(name="ps_tp", bufs=2, space="PSUM") as ps_tp_p:

                    def load_group_tiles(cg, half):
                        gth, xwh = [], []
                        for d in range(8):
                            t = gt_pool.tile([128, JC], dt.float16, tag="gth",
                                             name=f"gth{cg}_{half}_{d}")
                            nc.sync.dma_start(
                                t[:], g_ag[half][cg * D + d * 128:
                                                 cg * D + d * 128 + 128, :])
                            gth.append(t)
                        for k in range(4):
                            t = xw_pool.tile([128, O], dt.float16, tag="xwh",
                                             name=f"xwh{cg}_{half}_{k}")
                            nc.sync.dma_start(
                                t[:], xw_ag[half][cg * JC + k * 128:
                                                  cg * JC + k * 128 + 128, :])
                            xwh.append(t)
                        return gth, xwh

                    def emit_scores(gi, cg, half, s, tiles):
                        gth, _ = tiles
                        adj_t = adj_pool.tile([128, JC], dt.float8e5, tag="adj",
                                              name=f"adj{gi}_{s}")
                        nc.sync.dma_start(
                            adj_t[:], adj_d[s * 128:(s + 1) * 128,
                                            cg * NL + half * JC:
                                            cg * NL + half * JC + JC])
                        ps_sc = ps_sc_p.tile([128, JC], dt.float32, tag="ps_sc",
                                             name=f"ps_sc{gi}_{s}")
                        for d in range(8):
                            nc.tensor.matmul(ps_sc[:],
                                             fT[:, d * NL + s * 128:
                                                d * NL + s * 128 + 128],
                                             gth[d][:],
                                             start=(d == 0), stop=(d == 7))
                        return gi, s, ps_sc, adj_t, tiles

                    def emit_post(st):
                        gi, s, ps_sc, adj_t, tiles = st
                        _, xwh = tiles
                        p = gi % 2      # state parity
                        # t = relu(scores) + mask (masked entries -57344, so
                        # the exp below underflows them to exactly 0)
                        t_t = wk.tile([128, JC], dt.float32, tag="t",
                                      name=f"t{gi}_{s}")
                        nc.vector.scalar_tensor_tensor(
                            out=t_t[:], in0=ps_sc[:], scalar=0.0,
                            in1=adj_t[:], op0=ALU.max, op1=ALU.add)
                        nm_grp = tiny.tile([128, 1], dt.float32, tag="nm_grp",
                                           name=f"nmg{gi}_{s}")
                        nc.vector.tensor_reduce(out=nm_grp[:], in_=t_t[:],
                                                axis=mybir.AxisListType.X,
                                                op=ALU.max, negate=True)
                        nm_old = nm[s][:, p:p + 1]
                        nm_new = nm[s][:, 1 - p:2 - p]
                        nc.vector.tensor_tensor(out=nm_new, in0=nm_old,
                                                in1=nm_grp[:], op=ALU.min)
                        da = tiny.tile([128, 1], dt.float32, tag="da",
                                       name=f"da{gi}_{s}")
                        nc.vector.tensor_tensor(out=da[:], in0=nm_new,
                                                in1=nm_old, op=ALU.subtract)
                        alpha = tiny.tile([128, 1], dt.float32, tag="alpha",
                                          name=f"al{gi}_{s}")
                        nc.scalar.activation(alpha[:], da[:], AF.Exp)

                        # P = exp(t - m_run) in fp16, row sums via accum_out
                        l_grp = tiny.tile([128, 1], dt.float32, tag="l_grp",
                                          name=f"lg{gi}_{s}")
                        P16 = wk.tile([128, JC], dt.float16, tag="P",
                                      name=f"P{gi}_{s}")
                        nc.scalar.activation(P16[:], t_t[:], AF.Exp,
                                             bias=nm_new, scale=1.0,
                                             accum_out=l_grp[:])
                        # l_run update: l_new = l_old*alpha + l_grp
                        nc.vector.scalar_tensor_tensor(
                            out=lr[s][:, 1 - p:2 - p], in0=lr[s][:, p:p + 1],
                            scalar=alpha[:], in1=l_grp[:],
                            op0=ALU.mult, op1=ALU.add)

                        # 4 PE transposes into one PSUM bank, single copy out
                        tp = ps_tp_p.tile([128, JC], dt.float16, tag="tp",
                                          name=f"tp{gi}_{s}")
                        for k in range(4):
                            ks = slice(k * 128, k * 128 + 128)
                            nc.tensor.transpose(tp[:, ks], P16[:, ks], ident[:])
                        pt = wk.tile([128, JC], dt.float16, tag="pt",
                                     name=f"pt{gi}_{s}")
                        if s % 2 == 0:
                            nc.scalar.copy(pt[:], tp[:])
                        else:
                            nc.vector.tensor_copy(pt[:], tp[:])

                        ps_at = ps_at_p.tile([128, O], dt.float32, tag="ps_at",
                                             name=f"ps_at{gi}_{s}")
                        for k in range(4):
                            ks = slice(k * 128, k * 128 + 128)
                            for oc in range(2):
                                ocs = slice(oc * JC, oc * JC + JC)
                                nc.tensor.matmul(ps_at[:, ocs], pt[:, ks],
                                                 xwh[k][:, ocs],
                                                 start=(k == 0), stop=(k == 3))
                        # acc = acc*alpha + ps_at  (in-place)
                        for oc in range(2):
                            ocs = slice(oc * JC, oc * JC + JC)
                            nc.vector.scalar_tensor_tensor(
                                out=acc[s][:, ocs], in0=acc[s][:, ocs],
                                scalar=alpha[:], in1=ps_at[:, ocs],
                                op0=ALU.mult, op1=ALU.add)
                        if gi == 15:
                            # last j-group: finalize this strip now so the
                            # output DMA drains behind the remaining strips.
                            # The 1/l scale runs on the (idle) scalar engine.
                            rl = tiny.tile([128, 1], dt.float32, tag="rl",
                                           name=f"rl{s}")
                            nc.vector.reciprocal(rl[:], lr[s][:, 1 - p:2 - p])
                            o_sb = wk.tile([128, O], dt.float32, tag="o_sb",
                                           name=f"o_sb{s}")
                            nc.scalar.activation(o_sb[:], acc[s][:],
                                                 AF.Copy, scale=rl[:])
                            nc.sync.dma_start(out_d[s * 128:(s + 1) * 128, :],
                                              o_sb[:])

                    pending = None
                    gi = 0
                    for half in range(2):
                        for cg in range(N_CORES):
                            tiles = load_group_tiles(cg, half)
                            for s in range(8):     # i-strip
                                cur = emit_scores(gi, cg, half, s, tiles)
                                if pending is not None:
                                    emit_post(pending)
                                pending = cur
                            gi += 1
                    emit_post(pending)

    nc.compile()
    return nc


def _split16(a):
    hi = a.astype(np.float16)
    lo = (a.astype(np.float32) - hi.astype(np.float32)).astype(np.float16)
    return hi, lo


def _prep_inputs(x, adj, Wf, bf, Wg, bg, W, bW):
    ph1 = os.environ.get("PH1", "331")
    P1 = {"wf": int(ph1[0]), "wg": int(ph1[1]), "ww": int(ph1[2])}
    need_lo = max(P1.values()) >= 2
    x = np.asarray(x, dtype=np.float32)
    adj = np.asarray(adj)
    base = {}
    for wn, wv in (("wf", Wf), ("wg", Wg), ("ww", W)):
        hi, lo = _split16(np.asarray(wv, dtype=np.float32))
        base[wn] = hi
        if P1[wn] >= 3:
            base[f"{wn}_lo"] = lo
    base["bf"] = np.asarray(bf, dtype=np.float32).reshape(D, 1)
    base["bg"] = np.asarray(bg, dtype=np.float32).reshape(D, 1)
    base["bw"] = np.asarray(bW, dtype=np.float32).reshape(1, O)
    in_maps = []
    for c in range(N_CORES):
        rows = slice(c * NL, (c + 1) * NL)
        xT = np.ascontiguousarray(x[rows].T)
        xT_hi, xT_lo = _split16(xT)
        adjB = ((adj[rows].astype(np.int32) - 1) * 57344).astype(
            ml_dtypes.float8_e5m2)
        m = {"xT": xT_hi, "adj": adjB, **base}
        if need_lo:
            m["xT_lo"] = xT_lo
        in_maps.append(m)
    return in_maps


def run(inputs, trace=False):
    reps = int(os.environ.get("KERNEL_REPS", "1"))
    key = (f"nc{reps}_{os.environ.get('COLL_NONE', '0')}_"
           f"{os.environ.get('PH1', '331')}")
    if key not in _cache:
        _cache[key] = _build(reps=reps)
    nc = _cache[key]
    in_maps = _prep_inputs(**inputs)
    res = run_bass_kernel_spmd(nc, in_maps, list(range(N_CORES)), trace=trace)
    out = np.concatenate([res.results[c]["out"] for c in range(N_CORES)],
                         axis=0)
    return out, res


def kernel(**inputs) -> np.ndarray:
    out, _ = run(inputs, trace=False)
    return out


def bench(inputs, iters=6):
    """Wall-clock the NEFF execution with device-resident inputs."""
    import time
    import jax
    from jax.sharding import Mesh, PartitionSpec, NamedSharding
    from jax.experimental.shard_map import shard_map
    from concourse.bass2jax import (_bass_exec_p, install_neuronx_cc_hook,
                                    partition_id_tensor)

    reps = int(os.environ.get("KERNEL_REPS", "1"))
    key = (f"nc{reps}_{os.environ.get('COLL_NONE', '0')}_"
           f"{os.environ.get('PH1', '331')}")
    if key not in _cache:
        _cache[key] = _build(reps=reps)
    nc = _cache[key]
    install_neuronx_cc_hook()
    in_maps = _prep_inputs(**inputs)

    part_name = nc.partition_id_tensor.name if nc.partition_id_tensor else None
    in_names, out_names, out_avals, zero_outs = [], [], [], []
    for alloc in nc.m.functions[0].allocations:
        if not isinstance(alloc, mybir.MemoryLocationSet):
            continue
        name = alloc.memorylocations[0].name
        if alloc.kind == "ExternalInput":
            if name != part_name:
                in_names.append(name)
        elif alloc.kind == "ExternalOutput":
            out_names.append(name)
            shape = tuple(alloc.tensor_shape)
            npdt = mybir.dt.np(alloc.dtype)
            out_avals.append(jax.core.ShapedArray(shape, npdt))
            zero_outs.append(np.zeros(shape, npdt))
    n_params = len(in_names)
    all_names = in_names + out_names
    if part_name is not None:
        all_names = all_names + [part_name]

    def _body(*args):
        operands = list(args)
        if part_name is not None:
            operands.append(partition_id_tensor())
        outs = _bass_exec_p.bind(
            *operands,
            out_avals=tuple(out_avals),
            in_names=tuple(all_names),
            out_names=tuple(out_names),
            lowering_input_output_aliases=(),
            sim_require_finite=True,
            sim_require_nnan=True,
            nc=nc,
        )
        return tuple(outs)

    devices = jax.devices()[:N_CORES]
    mesh = Mesh(np.asarray(devices), ("core",))
    spec = PartitionSpec("core")
    n_all = n_params + len(out_names)
    fn = jax.jit(shard_map(_body, mesh=mesh, in_specs=(spec,) * n_all,
                           out_specs=(spec,) * len(out_names), check_rep=False),
                 keep_unused=True)
    concat_in = [np.concatenate([np.asarray(in_maps[c][n])
                                 for c in range(N_CORES)], axis=0)
                 for n in in_names]
    concat_zeros = [np.zeros((N_CORES * z.shape[0], *z.shape[1:]), z.dtype)
                    for z in zero_outs]
    sharding = NamedSharding(mesh, spec)
    dev_args = [jax.device_put(a, sharding) for a in concat_in + concat_zeros]
    # warmup
    r = fn(*dev_args)
    jax.block_until_ready(r)
    times = []
    for _ in range(iters):
        t0 = time.perf_counter()
        r = fn(*dev_args)
        jax.block_until_ready(r)
        times.append(time.perf_counter() - t0)
    ts = sorted(times)
    print("bench times ms:", " ".join(f"{t*1e3:.1f}" for t in ts))
    med = ts[len(ts) // 2]
    print(f"min {ts[0]*1e3:.2f}  p25 {ts[len(ts)//4]*1e3:.2f}  "
          f"median {med*1e3:.2f}")
    return med * 1e9
